# revision 1
# baseline (speedup 1.0000x reference)
"""Trainium2 Bass kernel for nn_AMIPRouterTrain (moe_routing).

Computes, for full inputs (N=4096 tokens, D=4096):
    weights = softmax(h_mask @ Wr + br)                      # [N, 8]
    cond    = concat([h_anchor, h_mask], -1)                 # [N, 8192]
    hid     = gelu(einsum('nd,kdf->knf', cond, W1) + b1)     # [8, N, 2048]
    eout    = einsum('knf,kfd->knd', hid, W2) + b2           # [8, N, 4096]
    delta   = einsum('knd,nk->nd', eout, weights)
    rel     = sigmoid(sum((h_mask@Wq+bq)*(h_anchor@Wk+bk), -1, keep) / sqrt(512))
    out     = delta * rel

Distribution over 8 NeuronCores (one trn2 chip):
  - Expert-parallel: core k owns expert k (W1[k]/W2[k] streamed from HBM),
    computes w_k-weighted expert output for ALL tokens, in bf16 with f32 PSUM
    accumulation.  The relevance gate is folded into the per-token scale
    (w_k * rel), so the cross-core combine is a plain sum.
  - Activations are NOT replicated: each core holds only its 1/8 token shard
    (8.4MB) and each 512-token block is AllGathered on-device right before
    use.  (This fleet's DRAM behaves like a small resident window backed by a
    ~5GB/s fault path; unique bytes touched per run dominate wall time, so
    the 67MB replicated activation matrix was the single largest cost.)
  - The q.k dot for the gate is sharded over the 512 projection dims (64 per
    core) and combined with a tiny AllReduce per 512-token block.
  - The weighted delta is combined with a bf16 ReduceScatter per 512-token
    block (8 blocks, pipelined against compute); core i receives rows
    [b*512+64*i, b*512+64*(i+1)) of each block and writes them to its output
    shard.  The host reassembles the full [4096, 4096] output and converts
    back to f32.

Everything core-dependent is prepared host-side (per-core in_maps): expert
weights, the token shard, Wr rolled by k so column 0 is always "my" expert,
Wq/Wk column shards.  The device graph is identical on all cores (SPMD).
"""

import os
import sys

for _p in ("/opt/trn_rl_repo", "/root/.axon_site/_ro/trn_rl_repo"):
    if os.path.isdir(_p) and _p not in sys.path:
        sys.path.insert(0, _p)

import numpy as np
import ml_dtypes

BF16 = ml_dtypes.bfloat16

# Problem dims (hardcoded per spec)
D = 4096          # d_model
NEXP = 8          # experts
DH = 2048         # expert hidden
DP = 512          # gate projection dim
NTOK = 4096       # tokens

N_CORES = 8
TB = 512              # tokens per block
NB = NTOK // TB       # 8 blocks
TT = TB // 128        # 4 token tiles per block
FT = DH // 128        # 16 f-tiles (stage-1 outputs / stage-2 contraction)
DC = (2 * D) // 128   # 64 contraction chunks for stage 1
DT = D // 512         # 8 d-tiles for stage 2
PS = DP // N_CORES    # 64 projection dims per core
RSQRT_DP = float(1.0 / np.sqrt(np.float32(DP)))

_PATCHED = False
_RUNNERS = {}


def _patch_drain():
    """This neuronxcc rejects instructions with >1-2 sem waits on the
    sequencer-only Drain at TileContext exit.  Split the waits across
    chained NOPs (sync engine, program order) — semantics preserved since
    every wait still precedes the final barrier."""
    global _PATCHED
    if _PATCHED:
        return
    import concourse.mybir as mybir
    import concourse.tile as tile
    from concourse.vector_clock import ScopedClock

    def _drain_and_barrier(self, tick_clock, wait_clock):
        drain_inst = self.nc.sync.drain()
        wait_clock.add_sem_waits(
            drain_inst.ins, ScopedClock({None: tick_clock.global_clock})
        )
        ins = drain_inst.ins
        si = ins.sync_info
        waits = list(si.on_wait)
        if len(waits) > 1:
            ins.sync_info = mybir.SyncInfo(
                on_wait=[waits[0]], on_update=list(si.on_update)
            )
            for w in waits[1:]:
                nop = self.nc.sync.nop(nofuse=True, hint="drain_wait_split")
                nop.ins.sync_info = mybir.SyncInfo(on_wait=[w], on_update=[])
        self.nc.all_engine_barrier()
        assert self.sems is not None
        popped = self.nc._tile_sem_poison_stack.pop()
        assert popped is self._sem_poison
        self.nc.clear_and_free_semaphores(list(self.sems.allocated().values()))
        self.nc.all_engine_barrier()

    tile.TileContext._drain_and_barrier = _drain_and_barrier
    _PATCHED = True


def build_graph(repeat: int = 1):
    """Build the SPMD Bass graph (same on all 8 cores)."""
    _patch_drain()
    import concourse.bacc as bacc
    import concourse.mybir as mybir
    import concourse.tile as tile

    f32 = mybir.dt.float32
    bf = mybir.dt.bfloat16
    AF = mybir.ActivationFunctionType
    X = mybir.AxisListType.X
    RG = [list(range(N_CORES))]

    nc = bacc.Bacc(num_devices=N_CORES)

    SH = TB // N_CORES  # 64 tokens contributed per core per block
    # per-core token shard: [block][d-chunk][d-in-chunk][token-in-shard]
    xsh = nc.declare_dram_parameter("xsh", [NB, DC, 128, SH], bf, isOutput=False)
    w1 = nc.declare_dram_parameter("w1", [FT, DC, 128, 128], bf, isOutput=False)
    w2 = nc.declare_dram_parameter("w2", [DH, D], bf, isOutput=False)
    wrq = nc.declare_dram_parameter("wrq", [32, 128, 8 + PS], bf, isOutput=False)
    wkp = nc.declare_dram_parameter("wk", [32, 128, PS], bf, isOutput=False)
    b1p = nc.declare_dram_parameter("b1", [FT, 128], f32, isOutput=False)
    b2p = nc.declare_dram_parameter("b2", [1, D], bf, isOutput=False)
    brq = nc.declare_dram_parameter("brq", [1, 8 + PS], bf, isOutput=False)
    bkp = nc.declare_dram_parameter("bk", [1, PS], bf, isOutput=False)
    out = nc.declare_dram_parameter("out", [NTOK // N_CORES, D], bf, isOutput=True)

    with tile.TileContext(nc) as tc:
        with tc.tile_pool(name="res", bufs=1) as res, \
             tc.tile_pool(name="xp", bufs=1) as xp, \
             tc.tile_pool(name="w1p", bufs=2) as w1p, \
             tc.tile_pool(name="hidp", bufs=2 * FT) as hidp, \
             tc.tile_pool(name="w2p", bufs=3) as w2p, \
             tc.tile_pool(name="outp", bufs=4) as outp, \
             tc.tile_pool(name="sm", bufs=2) as sm, \
             tc.tile_pool(name="ps1pool", bufs=2, space="PSUM") as ps1pool, \
             tc.tile_pool(name="ps2pool", bufs=4, space="PSUM") as ps2pool, \
             tc.tile_pool(name="psqpool", bufs=2, space="PSUM") as psqpool, \
             tc.tile_pool(name="dramp", bufs=2, space="DRAM") as dramp:

            # --- resident small tensors ---
            wrq_sb = res.tile([128, 32, 8 + PS], bf, name="wrq_sb")
            nc.scalar.dma_start(wrq_sb[:], wrq.ap().rearrange("c p m -> p c m"))
            wk_sb = res.tile([128, 32, PS], bf, name="wk_sb")
            nc.scalar.dma_start(wk_sb[:], wkp.ap().rearrange("c p m -> p c m"))
            b1_sb = res.tile([128, FT], f32, name="b1_sb")
            nc.scalar.dma_start(b1_sb[:], b1p.ap().rearrange("t p -> p t"))
            b2_sb = res.tile([1, D], bf, name="b2_sb")
            nc.scalar.dma_start(b2_sb[:], b2p.ap())
            brq_sb = res.tile([1, 8 + PS], bf, name="brq_sb")
            nc.scalar.dma_start(brq_sb[:], brq.ap())
            bk_sb = res.tile([1, PS], bf, name="bk_sb")
            nc.scalar.dma_start(bk_sb[:], bkp.ap())
            ones_sb = res.tile([1, 128], bf, name="ones_sb")
            nc.vector.memset(ones_sb[:], 1.0)

            # --- stage the big params into internal DRAM (scratchpad) ---
            # PJRT param buffers on this fleet read at ~5GB/s (fault path);
            # internal DRAM reads at full HBM speed.  Touch each param byte
            # once here; the block loop below reads only the staged copies.
            xsh_s = dramp.tile([NB, DC, 128, SH], bf, name="xsh_s", tag="xsh_s",
                               bufs=1)
            for b in range(NB):
                nc.sync.dma_start(xsh_s[b], xsh[b])
            w1_s = dramp.tile([FT, DC, 128, 128], bf, name="w1_s", tag="w1_s",
                              bufs=1)
            for ft in range(FT):
                nc.scalar.dma_start(w1_s[ft], w1[ft])
            w2_s = dramp.tile([DH, D], bf, name="w2_s", tag="w2_s", bufs=1)
            for j in range(4):
                nc.gpsimd.dma_start(
                    w2_s[j * 512:(j + 1) * 512, :], w2[j * 512:(j + 1) * 512, :])
            w2src = w2_s.rearrange("(fc p) d -> p fc d", p=128)

            def issue_ag(b):
                # gather block b's activations from all cores:
                # core i contributes tokens [b*512+64i, b*512+64(i+1))
                ag_in = dramp.tile([DC, 128, SH], bf, name="ag_in", tag="ag_in")
                nc.sync.dma_start(ag_in[:], xsh_s[b])
                ag_out = dramp.tile([N_CORES, DC, 128, SH], bf,
                                    name="ag_out", tag="ag_out",
                                    addr_space="Shared")
                nc.gpsimd.collective_compute(
                    "AllGather", mybir.AluOpType.bypass, replica_groups=RG,
                    ins=[ag_in.opt()], outs=[ag_out.opt()],
                )
                return ag_out

            ag_next = issue_ag(0)
            steps = [(r, b) for r in range(repeat) for b in range(NB)]
            for si, (_r, b) in enumerate(steps):
                if True:
                    ag_out = ag_next
                    # [128, d-chunk, core, tok-in-shard]; (core, tok) = block
                    # token axis in global order
                    xb = xp.tile([128, DC, N_CORES, SH], bf, name="xb", tag="xb")
                    for i in range(N_CORES):
                        nc.sync.dma_start(
                            xb[:, :, i, :],
                            ag_out[i].rearrange("c p n -> p c n"))
                    if si + 1 < len(steps):
                        ag_next = issue_ag(steps[si + 1][1])

                    # ---- stage 1: hidT[ft] = gelu(W1^T x cond^T + b1) ----
                    hid = []
                    for ft in range(FT):
                        w1t = w1p.tile([128, DC, 128], bf, name="w1t", tag="w1t")
                        nc.sync.dma_start(w1t[:], w1_s[ft].rearrange("c p f -> p c f"))
                        ps1 = ps1pool.tile([128, TB], f32, name="ps1", tag="ps1")
                        for c in range(DC):
                            nc.tensor.matmul(
                                ps1[:], lhsT=w1t[:, c, :],
                                rhs=xb[:, c].rearrange("p i n -> p (i n)"),
                                start=(c == 0), stop=(c == DC - 1),
                            )
                        ht = hidp.tile([128, TB], bf, name="hid", tag="hid")
                        nc.scalar.activation(
                            ht[:], ps1[:], AF.Gelu, bias=b1_sb[:, ft:ft + 1]
                        )
                        hid.append(ht)

                    # ---- routing softmax + q.k gate partials per token tile ----
                    wcols = []
                    qksum = sm.tile([128, TT], f32, name="qksum", tag="qksum")
                    for t in range(TT):
                        # token subtile t = shard-slots [2t, 2t+2) of the block
                        def tokslice(c):
                            return xb[:, c, 2 * t:2 * t + 2, :].rearrange(
                                "p i n -> p (i n)")
                        # logits (8 cols, col 0 = my expert) and q-shard (64 cols)
                        pq = psqpool.tile([128, 8 + PS], f32, name="pq", tag="psq")
                        for c in range(32):
                            nc.tensor.matmul(
                                pq[:], lhsT=tokslice(32 + c), rhs=wrq_sb[:, c, :],
                                start=(c == 0), stop=False,
                            )
                        nc.tensor.matmul(
                            pq[:], lhsT=ones_sb[:], rhs=brq_sb[:],
                            start=False, stop=True,
                        )
                        pk = psqpool.tile([128, PS], f32, name="pk", tag="psq")
                        for c in range(32):
                            nc.tensor.matmul(
                                pk[:], lhsT=tokslice(c), rhs=wk_sb[:, c, :],
                                start=(c == 0), stop=False,
                            )
                        nc.tensor.matmul(
                            pk[:], lhsT=ones_sb[:], rhs=bk_sb[:],
                            start=False, stop=True,
                        )
                        # softmax over the 8 logits; keep only column 0
                        lg = sm.tile([128, 8], f32, name="lg", tag="lg")
                        nc.vector.tensor_copy(lg[:], pq[:, 0:8])
                        nmx = sm.tile([128, 1], f32, name="nmx", tag="nmx")
                        nc.vector.reduce_max(nmx[:], lg[:], axis=X, negate=True)
                        ex = sm.tile([128, 8], f32, name="ex", tag="ex")
                        ssum = sm.tile([128, 1], f32, name="ssum", tag="ssum")
                        nc.scalar.activation(
                            ex[:], lg[:], AF.Exp, bias=nmx[:], accum_out=ssum[:]
                        )
                        rcp = sm.tile([128, 1], f32, name="rcp", tag="rcp")
                        nc.vector.reciprocal(rcp[:], ssum[:])
                        wc = sm.tile([128, 1], f32, name="wc", tag="wc", bufs=8)
                        nc.vector.tensor_mul(wc[:], ex[:, 0:1], rcp[:])
                        wcols.append(wc)
                        # gate partial: sum over my 64 projection dims
                        qsb = sm.tile([128, PS], f32, name="qsb", tag="qsb")
                        nc.scalar.copy(qsb[:], pq[:, 8:8 + PS])
                        qk = sm.tile([128, PS], f32, name="qk", tag="qk")
                        nc.vector.tensor_mul(qk[:], qsb[:], pk[:])
                        nc.vector.reduce_sum(qksum[:, t:t + 1], qk[:], axis=X)

                    # tiny AllReduce of the gate partials for this block
                    ar_in = dramp.tile([128, TT], f32, name="ar_in", tag="ar_in")
                    nc.gpsimd.dma_start(ar_in[:], qksum[:])
                    ar_out = dramp.tile([128, TT], f32, name="ar_out",
                                        tag="ar_out", addr_space="Shared")
                    nc.gpsimd.collective_compute(
                        "AllReduce", mybir.AluOpType.add, replica_groups=RG,
                        ins=[ar_in.opt()], outs=[ar_out.opt()],
                    )
                    qkt = sm.tile([128, TT], f32, name="qkt", tag="qkt")
                    nc.gpsimd.dma_start(qkt[:], ar_out[:])
                    rel = sm.tile([128, TT], f32, name="rel", tag="rel")
                    nc.scalar.activation(rel[:], qkt[:], AF.Sigmoid, scale=RSQRT_DP)
                    wrelf = []
                    for t in range(TT):
                        wr_t = sm.tile([128, 1], f32, name="wrf", tag="wrf", bufs=8)
                        nc.vector.tensor_mul(wr_t[:], wcols[t][:], rel[:, t:t + 1])
                        wrelf.append(wr_t)

                    # ---- stage 2: delta_k = (w_k*rel) * (hidT^T @ W2 + b2) ----
                    rs_in = dramp.tile([TB, D], bf, name="rs_in", tag="rs_in")
                    for dti in range(DT):
                        dsl = slice(dti * 512, (dti + 1) * 512)
                        pst = [
                            ps2pool.tile([128, 512], f32, name="ps2", tag="ps2")
                            for _ in range(TT)
                        ]
                        for fh in range(2):
                            w2h = w2p.tile([128, 8, 512], bf, name="w2h", tag="w2h")
                            nc.scalar.dma_start(
                                w2h[:], w2src[:, fh * 8:(fh + 1) * 8, dsl]
                            )
                            for f2i in range(8):
                                f2 = fh * 8 + f2i
                                for t in range(TT):
                                    nc.tensor.matmul(
                                        pst[t][:],
                                        lhsT=hid[f2][:, t * 128:(t + 1) * 128],
                                        rhs=w2h[:, f2i, :],
                                        start=(f2 == 0), stop=False,
                                    )
                        for t in range(TT):
                            nc.tensor.matmul(
                                pst[t][:], lhsT=ones_sb[:], rhs=b2_sb[:, dsl],
                                start=False, stop=True,
                            )
                            ob = outp.tile([128, 512], bf, name="ob", tag="ob")
                            nc.vector.tensor_scalar_mul(ob[:], pst[t][:], wrelf[t][:])
                            eng = nc.sync if (dti % 2 == 0) else nc.scalar
                            eng.dma_start(
                                rs_in[t * 128:(t + 1) * 128, dsl], ob[:]
                            )

                    # combine the 8 experts' weighted deltas; core i receives
                    # rows [64*i, 64*(i+1)) of this block
                    rs_out = dramp.tile([TB // N_CORES, D], bf, name="rs_out",
                                        tag="rs_out")
                    nc.gpsimd.collective_compute(
                        "ReduceScatter", mybir.AluOpType.add, replica_groups=RG,
                        ins=[rs_in.opt()], outs=[rs_out.opt()],
                    )
                    ob_sl = slice(b * (TB // N_CORES), (b + 1) * (TB // N_CORES))
                    nc.sync.dma_start(out.ap()[ob_sl, :], rs_out[:])

    nc.compile()
    return nc


class _Runner:
    def __init__(self, repeat: int):
        import jax
        from jax.sharding import Mesh, PartitionSpec
        from jax.experimental.shard_map import shard_map
        import concourse.mybir as mybir
        from concourse import bass2jax

        bass2jax.install_neuronx_cc_hook()
        nc = build_graph(repeat)
        self.nc = nc
        partition_name = (
            nc.partition_id_tensor.name if nc.partition_id_tensor else None
        )
        in_names, out_names, out_avals, zero_outs = [], [], [], []
        for alloc in nc.m.functions[0].allocations:
            if not isinstance(alloc, mybir.MemoryLocationSet):
                continue
            name = alloc.memorylocations[0].name
            if alloc.kind == "ExternalInput":
                if name != partition_name:
                    in_names.append(name)
            elif alloc.kind == "ExternalOutput":
                shape = tuple(alloc.tensor_shape)
                dtype = mybir.dt.np(alloc.dtype)
                out_names.append(name)
                out_avals.append(jax.core.ShapedArray(shape, dtype))
                zero_outs.append(np.zeros(shape, dtype))
        self.in_names = in_names
        self.out_names = out_names
        self.out_avals = out_avals
        self.zero_outs = zero_outs
        n_params = len(in_names)
        n_outs = len(out_avals)
        all_in = list(in_names) + list(out_names)
        if partition_name is not None:
            all_in.append(partition_name)

        def _body(*args):
            operands = list(args)
            if partition_name is not None:
                operands.append(bass2jax.partition_id_tensor())
            outs = bass2jax._bass_exec_p.bind(
                *operands,
                out_avals=tuple(out_avals),
                in_names=tuple(all_in),
                out_names=tuple(out_names),
                lowering_input_output_aliases=(),
                sim_require_finite=True,
                sim_require_nnan=True,
                nc=nc,
            )
            return tuple(outs)

        devices = jax.devices()[:N_CORES]
        assert len(devices) == N_CORES, f"need {N_CORES} cores, got {len(devices)}"
        mesh = Mesh(np.asarray(devices), ("core",))
        in_specs = (PartitionSpec("core"),) * (n_params + n_outs)
        out_specs = (PartitionSpec("core"),) * n_outs
        self.fn = jax.jit(
            shard_map(_body, mesh=mesh, in_specs=in_specs, out_specs=out_specs,
                      check_rep=False),
            keep_unused=True,
        )
        self._dev_zeros = None

    def dev_zeros(self):
        import jax
        if self._dev_zeros is None:
            self._dev_zeros = [
                jax.device_put(
                    np.zeros((N_CORES * z.shape[0], *z.shape[1:]), z.dtype)
                )
                for z in self.zero_outs
            ]
        return self._dev_zeros


def _get_runner(repeat: int = 1) -> "_Runner":
    if repeat not in _RUNNERS:
        _RUNNERS[repeat] = _Runner(repeat)
    return _RUNNERS[repeat]


def pack_inputs(inputs: dict) -> dict:
    """Per-core in_maps, concatenated along axis 0 (shard_map layout)."""
    h_anchor = np.asarray(inputs["h_anchor"], np.float32)
    h_mask = np.asarray(inputs["h_mask"], np.float32)
    Wr = np.asarray(inputs["Wr"], np.float32)
    br = np.asarray(inputs["br"], np.float32)
    W1 = np.asarray(inputs["W1"], np.float32)
    b1 = np.asarray(inputs["b1"], np.float32)
    W2 = np.asarray(inputs["W2"], np.float32)
    b2 = np.asarray(inputs["b2"], np.float32)
    Wq = np.asarray(inputs["Wq"], np.float32)
    bq = np.asarray(inputs["bq"], np.float32)
    Wk = np.asarray(inputs["Wk"], np.float32)
    bk = np.asarray(inputs["bk"], np.float32)

    xT = np.ascontiguousarray(
        np.concatenate([h_anchor.T, h_mask.T], axis=0)
    ).astype(BF16)                                             # [8192, 4096]
    SH = TB // N_CORES
    # [d-chunk, d-in-chunk, block, core, token-in-shard]
    xTr = xT.reshape(DC, 128, NB, N_CORES, SH)

    per = {k: [] for k in
           ("xsh", "w1", "w2", "wrq", "wk", "b1", "b2", "brq", "bk")}
    for k in range(N_CORES):
        psl = slice(PS * k, PS * (k + 1))
        per["xsh"].append(
            np.ascontiguousarray(xTr[:, :, :, k, :].transpose(2, 0, 1, 3))
        )                                                      # [NB, DC, 128, SH]
        per["w1"].append(
            np.ascontiguousarray(
                W1[k].reshape(DC, 128, FT, 128).transpose(2, 0, 1, 3)
            ).astype(BF16)
        )
        per["w2"].append(W2[k].astype(BF16))
        wr_k = np.roll(Wr, -k, axis=1)
        per["wrq"].append(
            np.ascontiguousarray(
                np.concatenate([wr_k, Wq[:, psl]], axis=1).reshape(32, 128, 8 + PS)
            ).astype(BF16)
        )
        per["wk"].append(
            np.ascontiguousarray(Wk[:, psl].reshape(32, 128, PS)).astype(BF16)
        )
        per["b1"].append(b1[k].reshape(FT, 128).astype(np.float32))
        per["b2"].append(b2[k][None].astype(BF16))
        per["brq"].append(
            np.concatenate([np.roll(br, -k), bq[psl]])[None].astype(BF16)
        )
        per["bk"].append(bk[psl][None].astype(BF16))
    return {k: np.concatenate(v, axis=0) for k, v in per.items()}


def unshard_output(out_concat: np.ndarray) -> np.ndarray:
    """[8*512, 4096] concat of per-core shards -> full [4096, 4096] f32.

    Core i's shard rows are ordered block-major: row b*64+j of core i is
    global token b*512 + i*64 + j."""
    per = out_concat.astype(np.float32).reshape(N_CORES, NB, TB // N_CORES, D)
    return np.ascontiguousarray(
        per.transpose(1, 0, 2, 3).reshape(NTOK, D)
    )


def kernel(**inputs) -> np.ndarray:
    import jax

    runner = _get_runner(repeat=1)
    arrs = pack_inputs(inputs)
    dev = [jax.device_put(arrs[n]) for n in runner.in_names]
    outs = runner.fn(*dev, *runner.dev_zeros())
    out_concat = np.asarray(outs[0])
    return unshard_output(out_concat).astype(np.float32)


if __name__ == "__main__":
    # tiny self-driven sanity run with random data
    rng = np.random.default_rng(0)
    inputs = {
        "h_anchor": rng.standard_normal((NTOK, D), np.float32),
        "h_mask": rng.standard_normal((NTOK, D), np.float32),
        "Wr": rng.standard_normal((D, NEXP), np.float32) / np.sqrt(D),
        "br": np.zeros(NEXP, np.float32),
        "W1": rng.standard_normal((NEXP, 2 * D, DH), np.float32) / np.sqrt(2 * D),
        "b1": np.zeros((NEXP, DH), np.float32),
        "W2": rng.standard_normal((NEXP, DH, D), np.float32) / np.sqrt(DH),
        "b2": np.zeros((NEXP, D), np.float32),
        "Wq": rng.standard_normal((D, DP), np.float32) / np.sqrt(D),
        "bq": np.zeros(DP, np.float32),
        "Wk": rng.standard_normal((D, DP), np.float32) / np.sqrt(D),
        "bk": np.zeros(DP, np.float32),
    }
    out = kernel(**inputs)
    print("out", out.shape, out.dtype, float(np.abs(out).mean()))

